# revision 19
# baseline (speedup 1.0000x reference)
"""GNN message-passing layer on 8 Trainium2 NeuronCores.

Computes out[i] = relu(U @ (sum_{j: adj[j,i]>0} x[j]) / deg_i) where
deg_i = sum_j adj[j,i], for a dense binary adjacency matrix.

Sharding: columns i of adj (target nodes) are row-sharded across the 8
cores (core g owns i in [g*2048, (g+1)*2048)); x and U are replicated.

Each core streams its 32MB fp8 adjacency shard once through the tensor
engine (the binary adjacency is exact in fp8e4, so the host stages it as
raw fp8 bytes and the DMA is a plain HWDGE copy — no cast anywhere):

  stage 1:  one fp8 DoubleRow matmul pass. The PE contracts TWO 128-row
            j-tiles per instruction at 2 moving cols/cycle (4x the bf16
            rate). The full 128 stationary columns hold a hi/lo split of
            x so one pass gives ~fp16-grade accuracy:
              cols   0..63 : x_hi[d]     (fp8 of x, all 64 dims)
              col       64 : 1.0         (accumulates deg_i)
              cols 65..127 : x_lo[d]*16  (fp8 of (x - fp8(x))*16, d<63)
            PSUM accumulates [128, 2048] fp32 over all 64 tile-pairs.
  stage 2:  out[i,:] = relu((agg_hi @ U^T + agg_lo @ U^T/16) * (1/deg_i))
            via ONE 128-row-contraction matmul per 128-node chunk with
            strided stationary + fused ACT relu/scale: the stationary is
            the full agg column block [hi|deg|lo] and the moving tensor
            is [U^T | 0 | U^T[0:63]/16] stacked on the partition axis, so
            the hi/lo recombine (and the deg row cancels) inside the
            contraction itself. (Accumulating two matmuls with different
            stationary base partitions into one PSUM group faults the
            hardware — this formulation avoids it and is cheaper.)
            A 65th moving column (basis vector at the deg row) drops each
            chunk's deg into PSUM col 64 already chunk-transposed, so the
            1/deg ACT scale comes from a [128,1] DVE reciprocal — no
            DRAM round-trip to re-layout deg.

DMA ring use: the adjacency stream alternates between the two HWDGE
rings (sync/SP and scalar/ACT) — one ring saturates at ~360 GB/s while
both together sustain ~410 GB/s. x''/U^T and the output store ride the
gpsimd (SWDGE) queue so consecutive invocations' streams don't queue
behind a prior epilogue's tail DMAs.

Dim 63 of x has no lo half (only 127 stationary cols remain after deg);
its error is ~fp8-grade but affects 1/64 dims -> ~5e-3 overall rel err,
within the 2e-2 gate.
"""

import sys

sys.path.insert(0, "/opt/trn_rl_repo")

import numpy as np
from contextlib import ExitStack

import concourse.bass as bass
import concourse.bacc as bacc
import concourse.mybir as mybir
import concourse.tile as tile
from concourse.bass_utils import run_bass_kernel_spmd

N_CORES = 8
P = 128

N_NODES = 16384
D = 64
SHARD = N_NODES // N_CORES  # 2048

# double-j-tiles ([256, SHARD] slabs of the adjacency shard) per dma_start
TPD2 = 2


def build_nc(n=N_NODES, d=D, shard=SHARD, tpd2=TPD2, reps=1):
    """reps > 1 repeats the whole kernel body inside one NEFF — used only to
    measure per-invocation time as a slope (the axon RPC overhead per launch
    is ~90ms, far above the kernel time)."""
    njt2 = n // (2 * P)  # number of j-tile PAIRS (DoubleRow k-groups)
    cg = shard // P  # stage-2 output chunks
    f32 = mybir.dt.float32
    f8 = mybir.dt.float8e4

    # Bacc (not raw Bass): its compile() splits multi-semaphore waits into
    # event semaphores — TRN2 ISA instructions fit only one wait.
    nc = bacc.Bacc()
    a_sh = nc.declare_dram_parameter("a_sh", [n, shard], f8, isOutput=False)
    xp = nc.declare_dram_parameter("xp", [P, njt2 * 2 * P], f8, isOutput=False)
    ut = nc.declare_dram_parameter("ut", [P, d + 1], f32, isOutput=False)
    out = nc.declare_dram_parameter("out_sh", [shard, d], f32, isOutput=True)

    with tile.TileContext(nc) as tc:
        with ExitStack() as ctx:
            constp = ctx.enter_context(tc.tile_pool(name="const", bufs=1))
            apool = ctx.enter_context(tc.tile_pool(name="adj", bufs=3))
            pspool = ctx.enter_context(tc.tile_pool(name="psum1", bufs=1, space="PSUM"))
            ps2pool = ctx.enter_context(tc.tile_pool(name="psum2", bufs=2, space="PSUM"))
            postp = ctx.enter_context(tc.tile_pool(name="post", bufs=1))

            def emit_body():
                # stationary: partition p holds x''[(2*t2+s)*128+p, c] at
                # free offset (t2, s, c); c = [x_lo*16 | 1 | x_hi] layout.
                x_sb = constp.tile([P, njt2, 2, P], f8, tag="x")
                nc.gpsimd.dma_start(
                    x_sb[:], xp[:, :].rearrange("p (t s c) -> p t s c", s=2, c=P)
                )
                ut_sb = constp.tile([P, d + 1], f32, tag="ut")
                nc.gpsimd.dma_start(ut_sb[:], ut[:, :])

                # stage 1: aggT = x''^T @ a over all j-tile pairs (DoubleRow)
                ps_agg = pspool.tile([P, shard], f32, tag="agg")
                for td in range(njt2 // tpd2):
                    src = a_sh[td * tpd2 * 2 * P : (td + 1) * tpd2 * 2 * P, :].rearrange(
                        "(t s p) c -> p t s c", p=P, s=2
                    )
                    a_t = apool.tile([P, tpd2, 2, shard], f8, tag="a")
                    # alternate the two HWDGE rings: one ring tops out at
                    # ~360 GB/s, both together sustain ~410 GB/s
                    eng = nc.sync if td % 2 == 0 else nc.scalar
                    eng.dma_start(a_t[:], src)
                    for tt in range(tpd2):
                        t2 = td * tpd2 + tt
                        # 256-wide out chunks: the DoubleRow moving AP is
                        # [128, 2, w] = 2w free elements and the PE caps the
                        # moving free dim at 512 (wider hangs the engine).
                        # start=True zeroes the full 2KB PSUM bank, so only
                        # the bank-aligned chunk carries it; the odd chunk
                        # inherits the bank's pending-zero.
                        for c0 in range(0, shard, 256):
                            w = min(256, shard - c0)
                            nc.tensor.matmul(
                                ps_agg[:, c0 : c0 + w],
                                x_sb[:, t2, :, :],
                                a_t[:, tt, :, c0 : c0 + w],
                                start=(t2 == 0 and c0 % 512 == 0),
                                stop=(t2 == njt2 - 1),
                                perf_mode=mybir.MatmulPerfMode.DoubleRow,
                                skip_group_check=(c0 % 512 != 0),
                            )

                # epilogue: rows 0..63 = agg_hi, row 64 = deg,
                # rows 65..127 = agg_lo*16
                agg_sb = postp.tile([P, shard], f32, tag="aggsb")
                half = shard // 2
                nc.vector.tensor_copy(agg_sb[:, 0:half], ps_agg[:, 0:half])
                nc.scalar.copy(agg_sb[:, half:shard], ps_agg[:, half:shard])

                out_sb = postp.tile([P, cg * d], f32, tag="out")
                rec_t = postp.tile([P, cg], f32, tag="rec")
                # i = m*cg + cpos: strided stationary picks every cg-th column
                agg_all = agg_sb[:, :].rearrange("dd (m c) -> dd c m", c=cg)
                for cpos in range(cg):
                    ps2 = ps2pool.tile([P, d + 1], f32, tag="p2")
                    nc.tensor.matmul(
                        ps2[:],
                        agg_all[:, cpos, :],
                        ut_sb[:, :],
                        start=True,
                        stop=True,
                    )
                    nc.vector.reciprocal(
                        rec_t[:, cpos : cpos + 1], ps2[:, d : d + 1]
                    )
                    nc.scalar.activation(
                        out_sb[:, cpos * d : (cpos + 1) * d],
                        ps2[:, 0:d],
                        mybir.ActivationFunctionType.Relu,
                        scale=rec_t[:, cpos : cpos + 1],
                    )
                nc.gpsimd.dma_start(
                    out[:, :].rearrange("(m c) dd -> m (c dd)", c=cg), out_sb[:]
                )

            for _rep in range(reps):
                emit_body()
    nc.compile()
    return nc


def make_host_inputs(x, adj_mat, U, n=N_NODES, d=D, shard=SHARD, n_cores=N_CORES):
    """Build per-core input maps from the full problem inputs (dtype/layout
    transforms only — all arithmetic on the data happens on-device)."""
    np8 = mybir.dt.np(mybir.dt.float8e4)
    njt2 = n // (2 * P)

    x = np.asarray(x, dtype=np.float32)
    U = np.asarray(U, dtype=np.float32)

    x_hi8 = x.astype(np8)
    x_lo8 = ((x - x_hi8.astype(np.float32)) * 16.0).astype(np8)
    xpp = np.zeros((n, P), np8)
    xpp[:, 0:d] = x_hi8
    xpp[:, d] = np.ones((), np.float32).astype(np8)
    xpp[:, d + 1 : 2 * d] = x_lo8[:, 0 : d - 1]
    xp_t = np.ascontiguousarray(
        xpp.reshape(njt2, 2, P, P).transpose(2, 0, 1, 3).reshape(P, njt2 * 2 * P)
    )

    ut_full = np.zeros((P, d + 1), np.float32)
    ut_full[0:d, 0:d] = U.T
    ut_full[d + 1 : 2 * d, 0:d] = U.T[0 : d - 1] / 16.0  # row d: 0 cancels deg
    ut_full[d, d] = 1.0  # deg-selector column -> PSUM col d holds deg_i

    adj8 = np.asarray(adj_mat).astype(np8)  # binary values: exact in fp8
    in_maps = []
    for g in range(n_cores):
        a_sh = np.ascontiguousarray(adj8[:, g * shard : (g + 1) * shard])
        in_maps.append({"a_sh": a_sh, "xp": xp_t, "ut": ut_full})
    return in_maps


_NC_CACHE = {}


def get_nc(**kw):
    key = tuple(sorted(kw.items()))
    if key not in _NC_CACHE:
        _NC_CACHE[key] = build_nc(**kw)
    return _NC_CACHE[key]


def kernel(x, adj_mat, U, **run_kw):
    """Full inputs in, full output out. Shards across 8 NeuronCores."""
    in_maps = make_host_inputs(x, adj_mat, U)
    nc = get_nc()
    res = run_bass_kernel_spmd(nc, in_maps, core_ids=list(range(N_CORES)), **run_kw)
    out = np.concatenate(
        [res.results[g]["out_sh"] for g in range(N_CORES)], axis=0
    ).astype(np.float32)
    if run_kw:
        kernel.last_result = res
    return out


# revision 20
# speedup vs baseline: 1.1960x; 1.1960x over previous
"""GNN message-passing layer on 8 Trainium2 NeuronCores.

Computes out[i] = relu(U @ (sum_{j: adj[j,i]>0} x[j]) / deg_i) where
deg_i = sum_j adj[j,i], for a dense binary adjacency matrix.

Sharding: columns i of adj (target nodes) are row-sharded across the 8
cores (core g owns i in [g*2048, (g+1)*2048)); x and U are replicated.

Each core streams its 32MB fp8 adjacency shard once through the tensor
engine (the binary adjacency is exact in fp8e4, so the host stages it as
raw fp8 bytes and the DMA is a plain HWDGE copy — no cast anywhere):

  stage 1:  one fp8 DoubleRow matmul pass. The PE contracts TWO 128-row
            j-tiles per instruction at 2 moving cols/cycle (4x the bf16
            rate). The full 128 stationary columns hold a hi/lo split of
            x so one pass gives ~fp16-grade accuracy:
              cols   0..63 : x_hi[d]     (fp8 of x, all 64 dims)
              col       64 : 1.0         (accumulates deg_i)
              cols 65..127 : x_lo[d]*16  (fp8 of (x - fp8(x))*16, d<63)
            PSUM accumulates [128, 2048] fp32 over all 64 tile-pairs.
  stage 2:  out[i,:] = relu((agg_hi @ U^T + agg_lo @ U^T/16) * (1/deg_i))
            via ONE 128-row-contraction matmul per 128-node chunk with
            strided stationary + fused ACT relu/scale: the stationary is
            the full agg column block [hi|deg|lo] and the moving tensor
            is [U^T | 0 | U^T[0:63]/16] stacked on the partition axis, so
            the hi/lo recombine (and the deg row cancels) inside the
            contraction itself. (Accumulating two matmuls with different
            stationary base partitions into one PSUM group faults the
            hardware — this formulation avoids it and is cheaper.)
            A 65th moving column (basis vector at the deg row) drops each
            chunk's deg into PSUM col 64 already chunk-transposed, so the
            1/deg ACT scale comes from a [128,1] DVE reciprocal — no
            DRAM round-trip to re-layout deg.

DMA ring use: the adjacency stream alternates between the two HWDGE
rings (sync/SP and scalar/ACT) — one ring saturates at ~360 GB/s while
both together sustain ~410 GB/s. x''/U^T and the output store ride the
gpsimd (SWDGE) queue so consecutive invocations' streams don't queue
behind a prior epilogue's tail DMAs.

Dim 63 of x has no lo half (only 127 stationary cols remain after deg);
its error is ~fp8-grade but affects 1/64 dims -> ~5e-3 overall rel err,
within the 2e-2 gate.
"""

import sys

sys.path.insert(0, "/opt/trn_rl_repo")

import numpy as np
from contextlib import ExitStack

import concourse.bass as bass
import concourse.bacc as bacc
import concourse.mybir as mybir
import concourse.tile as tile
from concourse.bass_utils import run_bass_kernel_spmd

N_CORES = 8
P = 128

N_NODES = 16384
D = 64
SHARD = N_NODES // N_CORES  # 2048

# double-j-tiles ([256, SHARD] slabs of the adjacency shard) per dma_start
TPD2 = 2


def build_nc(n=N_NODES, d=D, shard=SHARD, tpd2=TPD2, reps=1):
    """reps > 1 repeats the whole kernel body inside one NEFF — used only to
    measure per-invocation time as a slope (the axon RPC overhead per launch
    is ~90ms, far above the kernel time)."""
    njt2 = n // (2 * P)  # number of j-tile PAIRS (DoubleRow k-groups)
    cg = shard // P  # stage-2 output chunks
    f32 = mybir.dt.float32
    f8 = mybir.dt.float8e4

    # Bacc (not raw Bass): its compile() splits multi-semaphore waits into
    # event semaphores — TRN2 ISA instructions fit only one wait.
    nc = bacc.Bacc()
    a_sh = nc.declare_dram_parameter("a_sh", [n, shard], f8, isOutput=False)
    xp = nc.declare_dram_parameter("xp", [P, njt2 * 2 * P], f8, isOutput=False)
    ut = nc.declare_dram_parameter("ut", [P, d + 1], f32, isOutput=False)
    out = nc.declare_dram_parameter("out_sh", [shard, d], f32, isOutput=True)

    with tile.TileContext(nc) as tc:
        with ExitStack() as ctx:
            constp = ctx.enter_context(tc.tile_pool(name="const", bufs=1))
            apool = ctx.enter_context(tc.tile_pool(name="adj", bufs=3))
            pspool = ctx.enter_context(tc.tile_pool(name="psum1", bufs=1, space="PSUM"))
            ps2pool = ctx.enter_context(tc.tile_pool(name="psum2", bufs=2, space="PSUM"))
            postp = ctx.enter_context(tc.tile_pool(name="post", bufs=1))

            def emit_body():
                # stationary: partition p holds x''[(2*t2+s)*128+p, c] at
                # free offset (t2, s, c); c = [x_lo*16 | 1 | x_hi] layout.
                x_sb = constp.tile([P, njt2, 2, P], f8, tag="x")
                nc.gpsimd.dma_start(
                    x_sb[:], xp[:, :].rearrange("p (t s c) -> p t s c", s=2, c=P)
                )
                ut_sb = constp.tile([P, d + 1], f32, tag="ut")
                nc.gpsimd.dma_start(ut_sb[:], ut[:, :])

                # stage 1: aggT = x''^T @ a over all j-tile pairs (DoubleRow)
                ps_agg = pspool.tile([P, shard], f32, tag="agg")
                for td in range(njt2 // tpd2):
                    src = a_sh[td * tpd2 * 2 * P : (td + 1) * tpd2 * 2 * P, :].rearrange(
                        "(t s p) c -> p t s c", p=P, s=2
                    )
                    a_t = apool.tile([P, tpd2, 2, shard], f8, tag="a")
                    # alternate the two HWDGE rings: one ring tops out at
                    # ~360 GB/s, both together sustain ~410 GB/s
                    eng = nc.sync if td % 2 == 0 else nc.scalar
                    eng.dma_start(a_t[:], src)
                    for tt in range(tpd2):
                        t2 = td * tpd2 + tt
                        # 256-wide out chunks: the DoubleRow moving AP is
                        # [128, 2, w] = 2w free elements and the PE caps the
                        # moving free dim at 512 (wider hangs the engine).
                        # start=True zeroes the full 2KB PSUM bank, so only
                        # the bank-aligned chunk carries it; the odd chunk
                        # inherits the bank's pending-zero.
                        for c0 in range(0, shard, 256):
                            w = min(256, shard - c0)
                            nc.tensor.matmul(
                                ps_agg[:, c0 : c0 + w],
                                x_sb[:, t2, :, :],
                                a_t[:, tt, :, c0 : c0 + w],
                                start=(t2 == 0 and c0 % 512 == 0),
                                stop=(t2 == njt2 - 1),
                                perf_mode=mybir.MatmulPerfMode.DoubleRow,
                                skip_group_check=(c0 % 512 != 0),
                            )

                # epilogue: rows 0..63 = agg_hi, row 64 = deg,
                # rows 65..127 = agg_lo*16
                # epilogue compute stays entirely on DVE: the ACT sequencer
                # carries half the adjacency stream, so any ACT compute here
                # would stall the next invocation's scalar-ring DMAs.
                agg_sb = postp.tile([P, shard], f32, tag="aggsb")
                nc.vector.tensor_copy(agg_sb[:], ps_agg[:])

                out_sb = postp.tile([P, cg * d], f32, tag="out")
                rec_t = postp.tile([P, cg], f32, tag="rec")
                # i = m*cg + cpos: strided stationary picks every cg-th column
                agg_all = agg_sb[:, :].rearrange("dd (m c) -> dd c m", c=cg)
                for cpos in range(cg):
                    ps2 = ps2pool.tile([P, d + 1], f32, tag="p2")
                    nc.tensor.matmul(
                        ps2[:],
                        agg_all[:, cpos, :],
                        ut_sb[:, :],
                        start=True,
                        stop=True,
                    )
                    nc.vector.reciprocal(
                        rec_t[:, cpos : cpos + 1], ps2[:, d : d + 1]
                    )
                    # out = max(ps2 * (1/deg), 0) fused in one DVE op
                    nc.vector.tensor_scalar(
                        out_sb[:, cpos * d : (cpos + 1) * d],
                        ps2[:, 0:d],
                        rec_t[:, cpos : cpos + 1],
                        0.0,
                        mybir.AluOpType.mult,
                        mybir.AluOpType.max,
                    )
                nc.gpsimd.dma_start(
                    out[:, :].rearrange("(m c) dd -> m (c dd)", c=cg), out_sb[:]
                )

            for _rep in range(reps):
                emit_body()
    nc.compile()
    return nc


def make_host_inputs(x, adj_mat, U, n=N_NODES, d=D, shard=SHARD, n_cores=N_CORES):
    """Build per-core input maps from the full problem inputs (dtype/layout
    transforms only — all arithmetic on the data happens on-device)."""
    np8 = mybir.dt.np(mybir.dt.float8e4)
    njt2 = n // (2 * P)

    x = np.asarray(x, dtype=np.float32)
    U = np.asarray(U, dtype=np.float32)

    x_hi8 = x.astype(np8)
    x_lo8 = ((x - x_hi8.astype(np.float32)) * 16.0).astype(np8)
    xpp = np.zeros((n, P), np8)
    xpp[:, 0:d] = x_hi8
    xpp[:, d] = np.ones((), np.float32).astype(np8)
    xpp[:, d + 1 : 2 * d] = x_lo8[:, 0 : d - 1]
    xp_t = np.ascontiguousarray(
        xpp.reshape(njt2, 2, P, P).transpose(2, 0, 1, 3).reshape(P, njt2 * 2 * P)
    )

    ut_full = np.zeros((P, d + 1), np.float32)
    ut_full[0:d, 0:d] = U.T
    ut_full[d + 1 : 2 * d, 0:d] = U.T[0 : d - 1] / 16.0  # row d: 0 cancels deg
    ut_full[d, d] = 1.0  # deg-selector column -> PSUM col d holds deg_i

    adj8 = np.asarray(adj_mat).astype(np8)  # binary values: exact in fp8
    in_maps = []
    for g in range(n_cores):
        a_sh = np.ascontiguousarray(adj8[:, g * shard : (g + 1) * shard])
        in_maps.append({"a_sh": a_sh, "xp": xp_t, "ut": ut_full})
    return in_maps


_NC_CACHE = {}


def get_nc(**kw):
    key = tuple(sorted(kw.items()))
    if key not in _NC_CACHE:
        _NC_CACHE[key] = build_nc(**kw)
    return _NC_CACHE[key]


def kernel(x, adj_mat, U, **run_kw):
    """Full inputs in, full output out. Shards across 8 NeuronCores."""
    in_maps = make_host_inputs(x, adj_mat, U)
    nc = get_nc()
    res = run_bass_kernel_spmd(nc, in_maps, core_ids=list(range(N_CORES)), **run_kw)
    out = np.concatenate(
        [res.results[g]["out_sh"] for g in range(N_CORES)], axis=0
    ).astype(np.float32)
    if run_kw:
        kernel.last_result = res
    return out
